# revision 25
# baseline (speedup 1.0000x reference)
"""Chamfer distance (L2) Bass kernel for 8 TRN2 NeuronCores.

Problem: xyz1 [B=8, N=8192, 3] f32, xyz2 [B=8, M=8192, 3] f32.
  d[b, n, m] = |xyz1[b,n] - xyz2[b,m]|^2
  dist1[b, n] = min_m d[b, n, m];  dist2[b, m] = min_n d[b, n, m]

Sharding: data-parallel over batch — core b handles batch b (B == n_cores == 8).
No collectives; outputs are gathered on the host.

Per-core algorithm — single pass over the distance matrix:
  d = x2[n] + y2[m] - 2*x.y is produced tile-by-tile by ONE TensorE matmul per
  512-col chunk via an augmented K=13 bf16 contraction: hi/lo splits of the
  coordinates give ~fp16-accurate cross terms at full bf16 PE rate, and
  ones-rows fold the squared-norm offsets in, so fp32 PSUM tiles hold finished
  d values.

  Drain + reduction, per row-tile i (one [128, 8192] stripe of d):
    - staging: ScalarE activation-copies (fp32 PSUM -> fp16 SBUF) into one
      [128, 8192] cp tile, two copies per 4096-col PSUM round. One persistent
      full-PSUM tile + subtile deps let the next round's matmuls start as
      soon as the copy covering their chunk finishes.
    - column mins (dist2): one DVE tensor_tensor min of cp into a parity
      accumulator acc2h[i % 2] (all-16-bit => fast DVE mode); plain
      tensor_copy on each accumulator's first use (kills the memset).
    - row mins (dist1): a tt-min halving tree (8192 -> 4096 -> 2048 -> 1024)
      plus a final tensor_reduce to rowp[:, i]. The tree is SOFTWARE
      PIPELINED across row-tiles (level L of row-tile i-L is emitted at
      iteration i) because a DVE op that depends on the immediately
      preceding DVE op pays a ~0.75us pipeline bubble on HW.
  GPSIMD is useless here: walrus rejects every two-tensor ALU op (and all
  PSUM access) on Pool. tensor_tensor_scan runs at ~2.6ns/step on DVE and
  tensor_tensor_reduce faults at runtime, hence the tt tree.
  dist2's final min over the 128 partition lanes of acc2 uses PE transposes
  of 128x128 blocks + batched free-axis reduce_min.
"""

import sys

if "/opt/trn_rl_repo" not in sys.path:
    sys.path.insert(0, "/opt/trn_rl_repo")

import numpy as np
import ml_dtypes

import concourse.bass as bass  # noqa: F401
import concourse.mybir as mybir
import concourse.tile as tile
from concourse import bacc
from concourse.bass_utils import run_bass_kernel_spmd
from concourse.masks import make_identity

BF16 = ml_dtypes.bfloat16

B = 8
N = 8192
M = 8192
P = 128  # output rows per tile (partition dim)
K = 13  # augmented contraction rows
BIG = 60000.0  # min-identity; finite in fp16, >> any squared distance here
ST = "float16"  # staging/accumulator dtype: 16-bit for DVE 2x mode

_NC_CACHE = {}


def _emit_transposed(tc, nc, pool, vec_sb, ident, out_dram):
    """vec_sb [P, n_blk] fp16 holds out[i*P + p] at [p, i]. PE-transpose to
    [n_blk, P], cast-copy to fp32, and DMA out contiguously (the direct
    [p, i]-strided DMA would scatter 4-byte elements)."""
    n_blk = vec_sb.shape[1]
    st = getattr(mybir.dt, ST)
    with tc.tile_pool(name="psum_o", bufs=1, space="PSUM") as psum_o:
        pt = psum_o.tile([n_blk, P], st)
        nc.tensor.transpose(pt[:, :], vec_sb[:, :], ident[:, :])
        ot = pool.tile([n_blk, P], mybir.dt.float32, tag="out_t")
        nc.vector.tensor_copy(ot[:, :], pt[:, :])
        nc.sync.dma_start(
            out=out_dram.ap().rearrange("(i p) -> i p", p=P), in_=ot[:, :]
        )


def _part_min(tc, nc, pool, acc, ident, osb, name):
    """Min over the 128 partitions of acc -> osb[P, n_blk] via PE transposes
    (4 blocks per PSUM bank) + batched free-axis reduce_min."""
    m_len = acc.shape[1]
    n_blk = m_len // P
    st = getattr(mybir.dt, ST)
    with tc.tile_pool(name=name, bufs=2, space="PSUM") as psum_t:
        for t0 in range(0, n_blk, 8):
            nb = min(8, n_blk - t0)
            pst = psum_t.tile([P, nb * P], st)
            for j in range(nb):
                nc.tensor.transpose(
                    pst[:, j * P : (j + 1) * P],
                    acc[:, (t0 + j) * P : (t0 + j + 1) * P],
                    ident[:, :],
                )
            nc.vector.tensor_reduce(
                out=osb[:, t0 : t0 + nb],
                in_=pst[:, :].rearrange("p (b f) -> p b f", b=nb),
                axis=mybir.AxisListType.X,
                op=mybir.AluOpType.min,
            )


def build_nc(
    n,
    m,
    mm_free=512,
    ps_group=2048,
    reps=1,
    cp_bufs=4,
):
    """Build + compile the per-core Bass program (SPMD, same on all cores).

    reps>1 repeats the main pass (identical results — min is idempotent);
    used only for timing: kernel time = slope of wall time vs reps.

    Engine-cost notes (measured on HW via reps-slope microbenches):
      - fp16 tensor_tensor min: ~0.45 ns/output element;
      - ScalarE fp32->fp16 copy: ~0.66 ns/element;
      - tensor_reduce: ~0.85 ns/input element;
      - tensor_tensor_scan: ~2.6 ns/step (bad); tensor_tensor_reduce:
        faults at runtime (worse). Hence the row min is a tt-min tree.
      - a DVE op that depends on the IMMEDIATELY PRECEDING DVE op pays a
        ~0.75 us pipeline bubble — the tree below is software-pipelined
        across row-tiles so consecutive DVE ops are always independent.
    """
    sup = 2 * ps_group  # columns per PSUM round (the full-PSUM tile)
    st = getattr(mybir.dt, ST)
    n_tiles = n // P
    n_supers = m // sup

    nc = bacc.Bacc("TRN2", target_bir_lowering=False, debug=False)
    sx = nc.dram_tensor("sx", [K, n], mybir.dt.bfloat16, kind="ExternalInput")
    my = nc.dram_tensor("my", [K, m], mybir.dt.bfloat16, kind="ExternalInput")
    d1 = nc.dram_tensor("dist1", [n], mybir.dt.float32, kind="ExternalOutput")
    d2 = nc.dram_tensor("dist2", [m], mybir.dt.float32, kind="ExternalOutput")

    with tile.TileContext(nc) as tc:
        with tc.tile_pool(name="singles", bufs=1) as singles:
            sx_sb = singles.tile([K, n], mybir.dt.bfloat16)
            my_sb = singles.tile([K, m], mybir.dt.bfloat16)
            nc.sync.dma_start(out=sx_sb[:, :], in_=sx.ap())
            nc.sync.dma_start(out=my_sb[:, :], in_=my.ap())

            # Two alternating dist2 accumulators (i even/odd) keep successive
            # acc2 RAW hops ≥2 DVE ops apart; combined once at the end.
            acc2h = [
                singles.tile([P, m], st, name=f"acc2_{j}") for j in range(2)
            ]
            rowp = singles.tile([P, n_tiles], st)
            n_blk = m // P
            osb = singles.tile([P, n_blk], st)
            ident = singles.tile([P, P], st)
            make_identity(nc, ident[:, :])

            with (
                tc.tile_pool(name="psum", bufs=1, space="PSUM") as psum_pool,
                tc.tile_pool(name="cp", bufs=2 * cp_bufs) as cp_pool,
                tc.tile_pool(name="t4p", bufs=5) as t4_pool,
                tc.tile_pool(name="t2p", bufs=3) as t2_pool,
                tc.tile_pool(name="t1p", bufs=3) as t1_pool,
                tc.tile_pool(name="t0p", bufs=3) as t0_pool,
            ):
                import contextlib

                # One persistent full-PSUM tile: matmuls of the next round
                # only wait on the ScalarE copy that read their column chunk
                # (subtile deps), so PE pipelines behind the drains.
                ps = psum_pool.tile([P, sup], mybir.dt.float32)

                rep_ctx = (
                    tc.For_i(0, reps, 1) if reps > 1 else contextlib.nullcontext()
                )
                with rep_ctx:
                    # Software-pipelined row-min tree, emitted per 4096-col
                    # PSUM round so consumption of round 0 overlaps round 1's
                    # staging. Level L of row-tile i-L is emitted at
                    # iteration i, keeping consecutive DVE ops independent.
                    t4s, t2s, t1s, t0s = {}, {}, {}, {}

                    def row_stages(i):
                        if i - 1 in t4s:
                            t4a, t4b = t4s.pop(i - 1)
                            t2 = t2_pool.tile([P, m // 4], st, tag="t2")
                            nc.vector.tensor_tensor(
                                out=t2[:, :], in0=t4a[:, :], in1=t4b[:, :],
                                op=mybir.AluOpType.min,
                            )
                            t2s[i - 1] = t2
                        if i - 2 in t2s:
                            t2 = t2s.pop(i - 2)
                            t1 = t1_pool.tile([P, m // 8], st, tag="t1")
                            nc.vector.tensor_tensor(
                                out=t1[:, :], in0=t2[:, : m // 8],
                                in1=t2[:, m // 8 :], op=mybir.AluOpType.min,
                            )
                            t1s[i - 2] = t1
                        if i - 3 in t1s:
                            t1 = t1s.pop(i - 3)
                            t0 = t0_pool.tile([P, m // 16], st, tag="t0")
                            nc.vector.tensor_tensor(
                                out=t0[:, :], in0=t1[:, : m // 16],
                                in1=t1[:, m // 16 :], op=mybir.AluOpType.min,
                            )
                            t0s[i - 3] = t0
                        if i - 4 in t0s:
                            t0 = t0s.pop(i - 4)
                            nc.vector.tensor_reduce(
                                out=rowp[:, i - 4 : i - 3], in_=t0[:, :],
                                axis=mybir.AxisListType.X, op=mybir.AluOpType.min,
                            )

                    for i in range(n_tiles):
                        lhsT = sx_sb[:, i * P : (i + 1) * P]
                        acc2 = acc2h[i % 2]
                        t4pair = []
                        for s in range(n_supers):
                            cp = cp_pool.tile([P, sup], st, tag="cp")
                            for t in range(sup // mm_free):
                                nc.tensor.matmul(
                                    ps[:, t * mm_free : (t + 1) * mm_free],
                                    lhsT=lhsT,
                                    rhs=my_sb[:, s * sup + t * mm_free : s * sup + (t + 1) * mm_free],
                                    start=True,
                                    stop=True,
                                )
                            # two ScalarE copies per PSUM round: drain at
                            # half-tile granularity so PE overlaps
                            for h in range(2):
                                nc.scalar.copy(
                                    out=cp[:, h * ps_group : (h + 1) * ps_group],
                                    in_=ps[:, h * ps_group : (h + 1) * ps_group],
                                )
                            # dist1 level 0 for this round
                            t4 = t4_pool.tile([P, sup // 2], st, tag="t4")
                            nc.vector.tensor_tensor(
                                out=t4[:, :], in0=cp[:, : sup // 2],
                                in1=cp[:, sup // 2 :], op=mybir.AluOpType.min,
                            )
                            t4pair.append(t4)
                            # dist2 partial for this round's columns (copy on
                            # each accumulator's first use — removes memset)
                            sl = acc2[:, s * sup : (s + 1) * sup]
                            if i < 2:
                                nc.vector.tensor_copy(sl, cp[:, :])
                            else:
                                nc.vector.tensor_tensor(
                                    out=sl, in0=cp[:, :], in1=sl,
                                    op=mybir.AluOpType.min,
                                )
                        t4s[i] = t4pair
                        row_stages(i)
                    for i in range(n_tiles, n_tiles + 4):
                        row_stages(i)

            nc.vector.tensor_tensor(
                out=acc2h[0][:, :], in0=acc2h[0][:, :], in1=acc2h[1][:, :],
                op=mybir.AluOpType.min,
            )
            _part_min(tc, nc, singles, acc2h[0], ident, osb, "pm")
            _emit_transposed(tc, nc, singles, rowp, ident, d1)
            _emit_transposed(tc, nc, singles, osb, ident, d2)

    nc.compile()
    return nc


def get_nc(n=N, m=M, **kw):
    key = (n, m, tuple(sorted(kw.items())))
    if key not in _NC_CACHE:
        _NC_CACHE[key] = build_nc(n, m, **kw)
    return _NC_CACHE[key]


def _split_hi_lo(a):
    hi = a.astype(BF16)
    lo = (a - hi.astype(np.float32)).astype(BF16)
    return hi, lo


def _stat_rows(u, u2h, u2l):
    """Stationary-side augmented rows [13, len] for points u [len, 3] f32."""
    uh, ul = _split_hi_lo(u)
    out = np.empty((K, u.shape[0]), BF16)
    out[0:3] = uh.T
    out[3:6] = uh.T
    out[6:9] = ul.T
    out[9] = BF16(1.0)
    out[10] = BF16(1.0)
    out[11] = u2h
    out[12] = u2l
    return np.ascontiguousarray(out)


def _mov_rows(v, v2h, v2l):
    """Moving-side augmented rows [13, len] for points v [len, 3] f32."""
    vh, vl = _split_hi_lo(v)
    out = np.empty((K, v.shape[0]), BF16)
    out[0:3] = (-2.0 * vh.astype(np.float32)).astype(BF16).T
    out[3:6] = (-2.0 * vl.astype(np.float32)).astype(BF16).T
    out[6:9] = out[0:3]
    out[9] = v2h
    out[10] = v2l
    out[11] = BF16(1.0)
    out[12] = BF16(1.0)
    return np.ascontiguousarray(out)


def _prep_core_inputs(x, y):
    """Augmented bf16 matrices for one batch: core computes d[n-tile, m] tiles
    with x stationary and y moving; both reductions happen in the same pass."""
    x = np.asarray(x, np.float32)
    y = np.asarray(y, np.float32)
    x2 = np.sum(x.astype(np.float64) * x, axis=-1).astype(np.float32)
    y2 = np.sum(y.astype(np.float64) * y, axis=-1).astype(np.float32)
    x2h, x2l = _split_hi_lo(x2)
    y2h, y2l = _split_hi_lo(y2)
    return {
        "sx": _stat_rows(x, x2h, x2l),
        "my": _mov_rows(y, y2h, y2l),
    }


def kernel(xyz1, xyz2):
    xyz1 = np.asarray(xyz1, np.float32)
    xyz2 = np.asarray(xyz2, np.float32)
    b, n, _ = xyz1.shape
    m = xyz2.shape[1]
    assert b == B and n == N and m == M, (b, n, m)

    nc = get_nc(n, m)
    in_maps = [_prep_core_inputs(xyz1[i], xyz2[i]) for i in range(b)]
    res = run_bass_kernel_spmd(nc, in_maps, core_ids=list(range(b)))
    dist1 = np.stack([res.results[i]["dist1"] for i in range(b)]).astype(np.float32)
    dist2 = np.stack([res.results[i]["dist2"] for i in range(b)]).astype(np.float32)
    return dist1, dist2


# revision 32
# speedup vs baseline: 1.2485x; 1.2485x over previous
"""Chamfer distance (L2) Bass kernel for 8 TRN2 NeuronCores.

Problem: xyz1 [B=8, N=8192, 3] f32, xyz2 [B=8, M=8192, 3] f32.
  d[b, n, m] = |xyz1[b,n] - xyz2[b,m]|^2
  dist1[b, n] = min_m d[b, n, m];  dist2[b, m] = min_n d[b, n, m]

Sharding: data-parallel over batch — core b handles batch b (B == n_cores == 8).
No collectives; outputs are gathered on the host.

Per-core algorithm — single pass over the distance matrix:
  d = x2[n] + y2[m] - 2*x.y is produced tile-by-tile by ONE TensorE matmul per
  512-col chunk via an augmented K=13 bf16 contraction: hi/lo splits of the
  coordinates give ~fp16-accurate cross terms at full bf16 PE rate, and
  ones-rows fold the squared-norm offsets in, so fp32 PSUM tiles hold finished
  d values.

  Drain + reduction, per row-tile i (one [128, 8192] stripe of d):
    - staging: ScalarE activation-copies (fp32 PSUM -> fp16 SBUF) into one
      [128, 8192] cp tile, two copies per 4096-col PSUM round. One persistent
      full-PSUM tile + subtile deps let the next round's matmuls start as
      soon as the copy covering their chunk finishes.
    - column mins (dist2): one DVE tensor_tensor min of cp into a parity
      accumulator acc2h[i % 2] (all-16-bit => fast DVE mode); plain
      tensor_copy on each accumulator's first use (kills the memset).
    - row mins (dist1): a tt-min halving tree (8192 -> 4096 -> 2048 -> 1024)
      plus a final tensor_reduce to rowp[:, i]. The tree is SOFTWARE
      PIPELINED across row-tiles (level L of row-tile i-L is emitted at
      iteration i) because a DVE op that depends on the immediately
      preceding DVE op pays a ~0.75us pipeline bubble on HW.
  GPSIMD is useless here: walrus rejects every two-tensor ALU op (and all
  PSUM access) on Pool. tensor_tensor_scan runs at ~2.6ns/step on DVE and
  tensor_tensor_reduce faults at runtime, hence the tt tree.
  dist2's final min over the 128 partition lanes of acc2 uses PE transposes
  of 128x128 blocks + batched free-axis reduce_min.
"""

import sys

if "/opt/trn_rl_repo" not in sys.path:
    sys.path.insert(0, "/opt/trn_rl_repo")

import numpy as np
import ml_dtypes

import concourse.bass as bass  # noqa: F401
import concourse.mybir as mybir
import concourse.tile as tile
from concourse import bacc
from concourse.bass_utils import run_bass_kernel_spmd
from concourse.masks import make_identity

BF16 = ml_dtypes.bfloat16

B = 8
N = 8192
M = 8192
P = 128  # output rows per tile (partition dim)
K = 13  # augmented contraction rows
BIG = 60000.0  # min-identity; finite in fp16, >> any squared distance here
ST = "float16"  # staging/accumulator dtype: 16-bit for DVE 2x mode

_NC_CACHE = {}


def _emit_transposed(tc, nc, pool, vec_sb, ident, out_dram):
    """vec_sb [P, n_blk] fp16 holds out[i*P + p] at [p, i]. PE-transpose to
    [n_blk, P], cast-copy to fp32, and DMA out contiguously (the direct
    [p, i]-strided DMA would scatter 4-byte elements)."""
    n_blk = vec_sb.shape[1]
    st = getattr(mybir.dt, ST)
    with tc.tile_pool(name="psum_o", bufs=1, space="PSUM") as psum_o:
        pt = psum_o.tile([n_blk, P], st)
        nc.tensor.transpose(pt[:, :], vec_sb[:, :], ident[:, :])
        ot = pool.tile([n_blk, P], mybir.dt.float32, tag="out_t")
        nc.vector.tensor_copy(ot[:, :], pt[:, :])
        nc.sync.dma_start(
            out=out_dram.ap().rearrange("(i p) -> i p", p=P), in_=ot[:, :]
        )


def _part_min(tc, nc, pool, acc, ident, osb, name):
    """Min over the 128 partitions of acc -> osb[P, n_blk] via PE transposes
    (4 blocks per PSUM bank) + batched free-axis reduce_min."""
    m_len = acc.shape[1]
    n_blk = m_len // P
    st = getattr(mybir.dt, ST)
    with tc.tile_pool(name=name, bufs=2, space="PSUM") as psum_t:
        for t0 in range(0, n_blk, 8):
            nb = min(8, n_blk - t0)
            pst = psum_t.tile([P, nb * P], st)
            for j in range(nb):
                nc.tensor.transpose(
                    pst[:, j * P : (j + 1) * P],
                    acc[:, (t0 + j) * P : (t0 + j + 1) * P],
                    ident[:, :],
                )
            nc.vector.tensor_reduce(
                out=osb[:, t0 : t0 + nb],
                in_=pst[:, :].rearrange("p (b f) -> p b f", b=nb),
                axis=mybir.AxisListType.X,
                op=mybir.AluOpType.min,
            )


def build_nc(
    n,
    m,
    mm_free=512,
    ps_group=2048,
    reps=1,
    cp_bufs=4,
):
    """Build + compile the per-core Bass program (SPMD, same on all cores).

    reps>1 repeats the main pass (identical results — min is idempotent);
    used only for timing: kernel time = slope of wall time vs reps.

    Engine-cost notes (measured on HW via reps-slope microbenches):
      - fp16 tensor_tensor min: ~0.45 ns/output element;
      - ScalarE fp32->fp16 copy: ~0.66 ns/element;
      - tensor_reduce: ~0.85 ns/input element;
      - tensor_tensor_scan: ~2.6 ns/step (bad); tensor_tensor_reduce:
        faults at runtime (worse). Hence the row min is a tt-min tree.
      - a DVE op that depends on the IMMEDIATELY PRECEDING DVE op pays a
        ~0.75 us pipeline bubble — the tree below is software-pipelined
        across row-tiles so consecutive DVE ops are always independent.
    """
    sup = 2 * ps_group  # columns per PSUM round (the full-PSUM tile)
    st = getattr(mybir.dt, ST)
    n_tiles = n // P
    n_supers = m // sup

    nc = bacc.Bacc("TRN2", target_bir_lowering=False, debug=False)
    sx = nc.dram_tensor("sx", [K, n], mybir.dt.bfloat16, kind="ExternalInput")
    my = nc.dram_tensor("my", [K, m], mybir.dt.bfloat16, kind="ExternalInput")
    d1 = nc.dram_tensor("dist1", [n], mybir.dt.float32, kind="ExternalOutput")
    d2 = nc.dram_tensor("dist2", [m], mybir.dt.float32, kind="ExternalOutput")

    with tile.TileContext(nc) as tc:
        with tc.tile_pool(name="singles", bufs=1) as singles:
            sx_sb = singles.tile([K, n], mybir.dt.bfloat16)
            my_sb = singles.tile([K, m], mybir.dt.bfloat16)
            nc.sync.dma_start(out=sx_sb[:, :], in_=sx.ap())
            nc.sync.dma_start(out=my_sb[:, :], in_=my.ap())

            # Two alternating dist2 accumulators (i even/odd) keep successive
            # acc2 RAW hops ≥2 DVE ops apart; combined once at the end.
            acc2h = [
                singles.tile([P, m], st, name=f"acc2_{j}") for j in range(2)
            ]
            rowp = singles.tile([P, n_tiles], st)
            n_blk = m // P
            osb = singles.tile([P, n_blk], st)
            ident = singles.tile([P, P], st)
            make_identity(nc, ident[:, :])

            with (
                tc.tile_pool(name="psum", bufs=1, space="PSUM") as psum_pool,
                tc.tile_pool(name="cp", bufs=2 * cp_bufs) as cp_pool,
                tc.tile_pool(name="t4p", bufs=5) as t4_pool,
                tc.tile_pool(name="t2p", bufs=3) as t2_pool,
                tc.tile_pool(name="t1p", bufs=3) as t1_pool,
                tc.tile_pool(name="t0p", bufs=3) as t0_pool,
            ):
                import contextlib

                # One persistent full-PSUM tile: matmuls of the next round
                # only wait on the ScalarE copy that read their column chunk
                # (subtile deps), so PE pipelines behind the drains.
                ps = psum_pool.tile([P, sup], mybir.dt.float32)

                rep_ctx = (
                    tc.For_i(0, reps, 1) if reps > 1 else contextlib.nullcontext()
                )
                with rep_ctx:
                    # Software-pipelined row-min tree. Level 0 pairs columns
                    # ACROSS the two rounds' cp tiles (one 4096-out op
                    # replaces two per-round pair-mins + their combine).
                    # Level L of row-tile i-L is emitted at iteration i,
                    # keeping consecutive DVE ops independent.
                    cps, t4s, t2s, t1s, t0s = {}, {}, {}, {}, {}

                    def row_stages(i):
                        if i - 1 in cps:
                            cpa, cpb = cps.pop(i - 1)
                            t4 = t4_pool.tile([P, m // 2], st, tag="t4")
                            nc.vector.tensor_tensor(
                                out=t4[:, :], in0=cpa[:, :], in1=cpb[:, :],
                                op=mybir.AluOpType.min,
                            )
                            t4s[i - 1] = t4
                        if i - 2 in t4s:
                            t4 = t4s.pop(i - 2)
                            t2 = t2_pool.tile([P, m // 4], st, tag="t2")
                            nc.vector.tensor_tensor(
                                out=t2[:, :], in0=t4[:, : m // 4],
                                in1=t4[:, m // 4 :], op=mybir.AluOpType.min,
                            )
                            t2s[i - 2] = t2
                        if i - 3 in t2s:
                            t2 = t2s.pop(i - 3)
                            t1 = t1_pool.tile([P, m // 8], st, tag="t1")
                            nc.vector.tensor_tensor(
                                out=t1[:, :], in0=t2[:, : m // 8],
                                in1=t2[:, m // 8 :], op=mybir.AluOpType.min,
                            )
                            t1s[i - 3] = t1
                        if i - 4 in t1s:
                            t1 = t1s.pop(i - 4)
                            t0 = t0_pool.tile([P, m // 16], st, tag="t0")
                            nc.vector.tensor_tensor(
                                out=t0[:, :], in0=t1[:, : m // 16],
                                in1=t1[:, m // 16 :], op=mybir.AluOpType.min,
                            )
                            t0s[i - 4] = t0
                        if i - 5 in t0s:
                            t0 = t0s.pop(i - 5)
                            nc.vector.tensor_reduce(
                                out=rowp[:, i - 5 : i - 4], in_=t0[:, :],
                                axis=mybir.AxisListType.X, op=mybir.AluOpType.min,
                            )

                    for i in range(n_tiles):
                        lhsT = sx_sb[:, i * P : (i + 1) * P]
                        acc2 = acc2h[i % 2]
                        cppair = []
                        for s in range(n_supers):
                            cp = cp_pool.tile([P, sup], st, tag="cp")
                            for t in range(sup // mm_free):
                                nc.tensor.matmul(
                                    ps[:, t * mm_free : (t + 1) * mm_free],
                                    lhsT=lhsT,
                                    rhs=my_sb[:, s * sup + t * mm_free : s * sup + (t + 1) * mm_free],
                                    start=True,
                                    stop=True,
                                )
                            # two ScalarE copies per PSUM round: drain at
                            # half-tile granularity so the next round's
                            # matmuls overlap the second half's copy (one
                            # whole-round copy collapses the pipeline into
                            # strict mm<->copy alternation: 2x slower)
                            for h in range(2):
                                nc.scalar.copy(
                                    out=cp[:, h * ps_group : (h + 1) * ps_group],
                                    in_=ps[:, h * ps_group : (h + 1) * ps_group],
                                )
                            # dist2 partial for this round's columns (copy on
                            # each accumulator's first use — removes memset)
                            sl = acc2[:, s * sup : (s + 1) * sup]
                            if i < 2:
                                nc.vector.tensor_copy(sl, cp[:, :])
                            else:
                                nc.vector.tensor_tensor(
                                    out=sl, in0=cp[:, :], in1=sl,
                                    op=mybir.AluOpType.min,
                                )
                            cppair.append(cp)
                        cps[i] = cppair
                        row_stages(i)
                    for i in range(n_tiles, n_tiles + 5):
                        row_stages(i)

            nc.vector.tensor_tensor(
                out=acc2h[0][:, :], in0=acc2h[0][:, :], in1=acc2h[1][:, :],
                op=mybir.AluOpType.min,
            )
            _part_min(tc, nc, singles, acc2h[0], ident, osb, "pm")
            _emit_transposed(tc, nc, singles, rowp, ident, d1)
            _emit_transposed(tc, nc, singles, osb, ident, d2)

    nc.compile()
    return nc


def get_nc(n=N, m=M, **kw):
    key = (n, m, tuple(sorted(kw.items())))
    if key not in _NC_CACHE:
        _NC_CACHE[key] = build_nc(n, m, **kw)
    return _NC_CACHE[key]


def _split_hi_lo(a):
    hi = a.astype(BF16)
    lo = (a - hi.astype(np.float32)).astype(BF16)
    return hi, lo


def _stat_rows(u, u2h, u2l):
    """Stationary-side augmented rows [13, len] for points u [len, 3] f32."""
    uh, ul = _split_hi_lo(u)
    out = np.empty((K, u.shape[0]), BF16)
    out[0:3] = uh.T
    out[3:6] = uh.T
    out[6:9] = ul.T
    out[9] = BF16(1.0)
    out[10] = BF16(1.0)
    out[11] = u2h
    out[12] = u2l
    return np.ascontiguousarray(out)


def _mov_rows(v, v2h, v2l):
    """Moving-side augmented rows [13, len] for points v [len, 3] f32."""
    vh, vl = _split_hi_lo(v)
    out = np.empty((K, v.shape[0]), BF16)
    out[0:3] = (-2.0 * vh.astype(np.float32)).astype(BF16).T
    out[3:6] = (-2.0 * vl.astype(np.float32)).astype(BF16).T
    out[6:9] = out[0:3]
    out[9] = v2h
    out[10] = v2l
    out[11] = BF16(1.0)
    out[12] = BF16(1.0)
    return np.ascontiguousarray(out)


def _prep_core_inputs(x, y):
    """Augmented bf16 matrices for one batch: core computes d[n-tile, m] tiles
    with x stationary and y moving; both reductions happen in the same pass."""
    x = np.asarray(x, np.float32)
    y = np.asarray(y, np.float32)
    x2 = np.sum(x.astype(np.float64) * x, axis=-1).astype(np.float32)
    y2 = np.sum(y.astype(np.float64) * y, axis=-1).astype(np.float32)
    x2h, x2l = _split_hi_lo(x2)
    y2h, y2l = _split_hi_lo(y2)
    return {
        "sx": _stat_rows(x, x2h, x2l),
        "my": _mov_rows(y, y2h, y2l),
    }


def kernel(xyz1, xyz2):
    xyz1 = np.asarray(xyz1, np.float32)
    xyz2 = np.asarray(xyz2, np.float32)
    b, n, _ = xyz1.shape
    m = xyz2.shape[1]
    assert b == B and n == N and m == M, (b, n, m)

    nc = get_nc(n, m)
    in_maps = [_prep_core_inputs(xyz1[i], xyz2[i]) for i in range(b)]
    res = run_bass_kernel_spmd(nc, in_maps, core_ids=list(range(b)))
    dist1 = np.stack([res.results[i]["dist1"] for i in range(b)]).astype(np.float32)
    dist2 = np.stack([res.results[i]["dist2"] for i in range(b)]).astype(np.float32)
    return dist1, dist2
